# revision 11
# baseline (speedup 1.0000x reference)
"""Trainium2 Bass kernel for nn_Attention_17222818857675.

Full-input contract: kernel(**inputs) takes the complete tensors, shards
across 8 NeuronCores internally (batch x head-group), runs one SPMD NEFF,
and gathers the full [4, 2048, 1152] output.

Per-core work (b = core//2, g = core%2, heads g*8..g*8+8):
  phase 1: QKV projection in natural [token, dim] layout (bf16 matmuls,
           fp32 PSUM), fused RoPE + per-head RMSNorm (stats computed
           pre-RoPE -- rotation is norm-preserving), PE-transpose of k-hat
           into [dim, token] layout for the scores matmul.
  phase 2: per (q-chunk, head): scores S^T = khT.T @ qhT (bf16), exp on
           ScalarE (fp32 PSUM -> bf16 SBUF), P^T @ V via PE with a ones
           column appended to V giving the softmax denominator for free,
           normalize via partition-broadcast DMA + DVE multiply, then the
           output projection in fp32r. Host sums the two half-head partial
           projections per batch and adds b_proj.
"""

import os
import sys
import types
import numpy as np
import ml_dtypes

# ---------------------------------------------------------------- constants
B, N, C = 4, 2048, 1152
H, DH, HALF = 16, 72, 36
HPC = 8              # heads per core
CPC = HPC * DH       # 576 contraction dims per core
EPS = 1e-6
THETA = 10000.0
NT = N // 128        # 16 token tiles
NCCH = C // 128      # 9 contraction chunks for qkv
QKVC = 288           # qkv output chunk (6 chunks over 1728)
NQKV = (3 * CPC) // QKVC
NJ = 4               # q-chunks of 512
TQ = 512
ECH = 384            # proj output chunk (3 chunks over 1152)
PCB = 5              # proj contraction blocks of 128 (576 -> 4.5 -> 5)

_BF16 = ml_dtypes.bfloat16


# ------------------------------------------------------------------- shims
def _install_shims():
    """axon_hooks module (missing in image) + Tile tail-drain walrus fix."""
    try:
        import antenv.axon_hooks  # noqa: F401
    except ImportError:
        import antenv

        m = types.ModuleType("antenv.axon_hooks")
        m._hook = None
        m.set_axon_ntff_profile_hook = lambda h: setattr(m, "_hook", h)
        m.get_axon_ntff_profile_hook = lambda: m._hook
        sys.modules["antenv.axon_hooks"] = m
        antenv.axon_hooks = m
        try:
            from trn_agent_boot.trn_boot import _ntff_profile_via_ctypes

            so = "/opt/axon/libaxon_pjrt.so"
            if os.path.exists(so):
                hook = _ntff_profile_via_ctypes(so)
                if hook:
                    m.set_axon_ntff_profile_hook(hook)
        except Exception:
            pass

    import concourse.tile as tile

    if getattr(tile.TileContext, "_drain_patched", False):
        return

    def _patched(self, tick_clock, wait_clock):
        nc = self.nc
        gc = tick_clock.global_clock
        for proc, sem in self.sems.allocated().items():
            v = gc[proc]
            if v > 0:
                mult = 16 if sem.name.startswith("DMA") else 1
                nc.sync.wait_ge(sem, v * mult)
        nc.sync.drain()
        nc.all_engine_barrier()
        popped = nc._tile_sem_poison_stack.pop()
        assert popped is self._sem_poison
        nc.clear_and_free_semaphores(list(self.sems.allocated().values()))
        nc.all_engine_barrier()

    tile.TileContext._drain_and_barrier = _patched
    tile.TileContext._drain_patched = True


# ------------------------------------------------------------------ builder
_NC = None


def _build():
    global _NC
    if _NC is not None:
        return _NC
    _install_shims()
    import concourse.bass as bass
    import concourse.mybir as mybir
    import concourse.tile as tile
    from concourse import bacc
    from concourse.masks import make_identity

    f32 = mybir.dt.float32
    f32r = mybir.dt.float32r
    bf16 = mybir.dt.bfloat16
    AF = mybir.ActivationFunctionType
    ALU = mybir.AluOpType

    nc = bacc.Bacc(trn_type="TRN2")

    xT_d = nc.dram_tensor("xT", (128, NT, NCCH, 128), bf16, kind="ExternalInput")
    wqkv_d = nc.dram_tensor("wqkv", (128, NCCH, 3 * CPC), bf16, kind="ExternalInput")
    wproj_d = nc.dram_tensor("wproj", (128, PCB, C), bf16, kind="ExternalInput")
    cosq_d = nc.dram_tensor("cosq", (128, NT, DH), f32, kind="ExternalInput")
    sinq_d = nc.dram_tensor("sinq", (128, NT, DH), f32, kind="ExternalInput")
    cosk_d = nc.dram_tensor("cosk", (128, NT, DH), f32, kind="ExternalInput")
    sink_d = nc.dram_tensor("sink", (128, NT, DH), f32, kind="ExternalInput")
    y_d = nc.dram_tensor("y", (N, C), f32, kind="ExternalOutput")

    def APX(base, dims, extra_off=0):
        return bass.AP(tensor=base.tensor, offset=base.offset + extra_off, ap=dims)

    with tile.TileContext(nc) as tc:
        from contextlib import ExitStack

        with ExitStack() as ctx:
            persist = ctx.enter_context(tc.tile_pool(name="persist", bufs=1))
            khT = persist.tile([DH, HPC, N], bf16)           # k-hat transposed
            vaug = persist.tile([128, NT, HPC, 97], bf16)  # 72 v | 24 zero | ones at 96
            qhat = persist.tile([128, NT, CPC], bf16)        # q-hat natural
            wqkv = persist.tile([128, NCCH, 3 * CPC], bf16)
            wproj = persist.tile([128, PCB, C], bf16)
            cosq = persist.tile([128, NT, DH], f32)
            sinq = persist.tile([128, NT, DH], f32)
            cosk = persist.tile([128, NT, DH], f32)
            sink = persist.tile([128, NT, DH], f32)
            ident = persist.tile([128, 128], bf16)
            eps_q = persist.tile([128, 1], f32)
            eps_k = persist.tile([128, 1], f32)

            make_identity(nc, ident[:])
            nc.vector.memset(eps_q[:], DH * EPS)
            nc.vector.memset(eps_k[:], EPS)
            nc.gpsimd.memset(vaug[:, :, :, DH:97], 0.0)
            nc.gpsimd.memset(vaug[:, :, :, 96:97], 1.0)
            nc.sync.dma_start(wqkv[:], wqkv_d[:])
            nc.sync.dma_start(wproj[:], wproj_d[:])
            nc.sync.dma_start(cosq[:], cosq_d[:])
            nc.sync.dma_start(sinq[:], sinq_d[:])
            nc.sync.dma_start(cosk[:], cosk_d[:])
            nc.sync.dma_start(sink[:], sink_d[:])

            # ------------------------------------------------ phase 1
            with tc.tile_pool(name="p1", bufs=3) as p1, \
                 tc.tile_pool(name="p1s", bufs=2) as p1s, \
                 tc.tile_pool(name="qkps", bufs=6, space="PSUM") as qkps, \
                 tc.tile_pool(name="trps", bufs=2, space="PSUM") as trps:
                for it in range(NT):
                    xt = p1.tile([128, NCCH, 128], bf16, tag="xt")
                    nc.sync.dma_start(xt[:], xT_d[:, it])

                    qk = p1.tile([128, 2 * CPC], f32, tag="qk")
                    for nch in range(NQKV):
                        ps = qkps.tile([128, QKVC], f32, tag="qkvps")
                        for cch in range(NCCH):
                            nc.tensor.matmul(
                                ps[:],
                                lhsT=xt[:, cch, :],
                                rhs=wqkv[:, cch, nch * QKVC : (nch + 1) * QKVC],
                                start=(cch == 0),
                                stop=(cch == NCCH - 1),
                            )
                        if nch < 4:  # q,k chunks -> fp32 natural buffer (ScalarE)
                            nc.scalar.copy(qk[:, nch * QKVC : (nch + 1) * QKVC], ps[:])
                        else:  # v chunks -> bf16 vaug, strided per-head cols
                            h0 = (nch - 4) * 4
                            nc.scalar.copy(
                                vaug[:, it, h0 : h0 + 4, 0:DH],
                                ps[:].rearrange("p (h d) -> p h d", h=4),
                            )

                    # RMS stats (pre-RoPE; rotation preserves norms)
                    sq = p1s.tile([128, 2 * CPC], f32, tag="sq")
                    nc.scalar.activation(sq[:], qk[:], AF.Square)
                    ms = p1s.tile([128, 16], f32, tag="ms")
                    nc.vector.tensor_reduce(
                        ms[:], sq[:].rearrange("p (g d) -> p g d", g=16),
                        axis=mybir.AxisListType.X, op=ALU.add,
                    )
                    rms = p1s.tile([128, 16], f32, tag="rms")
                    # q: 1/sqrt(sum + DH*eps) also folds the DH**-0.5 score scale
                    nc.scalar.activation(rms[:, 0:8], ms[:, 0:8], AF.Sqrt, bias=eps_q[:])
                    # k: 1/sqrt(sum/DH + eps)
                    nc.scalar.activation(rms[:, 8:16], ms[:, 8:16], AF.Sqrt,
                                         bias=eps_k[:], scale=1.0 / DH)
                    alpha = p1s.tile([128, 16], f32, tag="alpha")
                    nc.vector.reciprocal(alpha[:], rms[:])

                    # RoPE + alpha scaling.  qk cols: q = [0:576), k = [576:1152)
                    def rope(base_off, cos_t, sin_t, alpha_sl, out_sl, eng):
                        tmp = p1s.tile([128, CPC], f32, tag="ropetmp%d" % base_off)
                        rot = p1s.tile([128, CPC], f32, tag="roterot%d" % base_off)
                        qk0 = qk[:, base_off : base_off + CPC]
                        p_tmp, p_qk = tmp[:].ap[0], qk0.ap[0]
                        p_cos, p_sin = cos_t.ap[0], sin_t.ap[0]
                        p_al, p_out = alpha_sl.ap[0], out_sl.ap[0]
                        # tmp[h,0:36] = x2 * (-sin) ; tmp[h,36:72] = x1 * (+sin)
                        eng.tensor_tensor(
                            APX(tmp[:], [p_tmp, [DH, HPC], [1, HALF]]),
                            APX(qk0, [p_qk, [DH, HPC], [1, HALF]], HALF),
                            APX(sin_t, [p_sin, [0, HPC], [1, HALF]]),
                            op=ALU.mult,
                        )
                        eng.tensor_tensor(
                            APX(tmp[:], [p_tmp, [DH, HPC], [1, HALF]], HALF),
                            APX(qk0, [p_qk, [DH, HPC], [1, HALF]]),
                            APX(sin_t, [p_sin, [0, HPC], [1, HALF]], HALF),
                            op=ALU.mult,
                        )
                        eng.tensor_tensor(
                            rot[:].rearrange("p (h d) -> p h d", h=HPC),
                            qk0.rearrange("p (h d) -> p h d", h=HPC),
                            APX(cos_t, [p_cos, [0, HPC], [1, DH]]),
                            op=ALU.mult,
                        )
                        eng.tensor_tensor(rot[:], rot[:], tmp[:], op=ALU.add)
                        eng.tensor_tensor(
                            out_sl.rearrange("p (h d) -> p h d", h=HPC),
                            rot[:].rearrange("p (h d) -> p h d", h=HPC),
                            APX(alpha_sl, [p_al, [1, HPC], [0, DH]]),
                            op=ALU.mult,
                        )

                    rope(0, cosq[:, it, :], sinq[:, it, :], alpha[:, 0:8],
                         qhat[:, it, :], nc.vector)
                    khat = p1s.tile([128, CPC], bf16, tag="khat")
                    rope(CPC, cosk[:, it, :], sink[:, it, :], alpha[:, 8:16],
                         khat[:], nc.gpsimd)

                    # PE-transpose k-hat per head -> khT bf16
                    for hb in (0, 4):
                        tp = trps.tile([DH, 4, 128], bf16, tag="ktr")
                        for h4 in range(4):
                            nc.tensor.transpose(
                                tp[:, h4, :],
                                khat[:, (hb + h4) * DH : (hb + h4 + 1) * DH],
                                ident[:],
                            )
                        nc.vector.tensor_copy(
                            khT[0:DH, hb : hb + 4, it * 128 : (it + 1) * 128], tp[:]
                        )

            # ------------------------------------------------ phase 2
            with tc.tile_pool(name="p2", bufs=2) as p2, \
                 tc.tile_pool(name="p2o", bufs=2) as p2o, \
                 tc.tile_pool(name="sps", bufs=2, space="PSUM") as sps, \
                 tc.tile_pool(name="pvps", bufs=2, space="PSUM") as pvps, \
                 tc.tile_pool(name="yps", bufs=1, space="PSUM") as yps, \
                 tc.tile_pool(name="qtps", bufs=1, space="PSUM") as qtps, \
                 tc.tile_pool(name="dram", bufs=1, space="DRAM") as dpool:
                rec_dram = dpool.tile([NJ, HPC, TQ], f32)
                for j in range(NJ):
                    # transpose q-hat for this q-chunk -> qT bf16 [72, 8, 512]
                    qT = p2.tile([DH, HPC, TQ], bf16, tag="qT")
                    for h in range(HPC):
                        tp = qtps.tile([DH, 4, 128], bf16, tag="qtr")
                        for ts in range(4):
                            it = j * 4 + ts
                            nc.tensor.transpose(
                                tp[:, ts, :],
                                qhat[:, it, h * DH : (h + 1) * DH],
                                ident[:],
                            )
                        nc.vector.tensor_copy(qT[0:DH, h, :], tp[:])

                    proj_in = p2.tile([128, PCB, TQ], bf16, tag="proj_in")
                    for h in range(HPC):
                        pv = pvps.tile([97, TQ], f32, tag="pv")
                        for gg in range(8):  # k-tile pairs
                            sp = sps.tile([128, 2, TQ], f32, tag="sp")
                            pbuf = p2o.tile([128, 2, TQ], bf16, tag="pbuf")
                            for ii in range(2):
                                i = gg * 2 + ii
                                nc.tensor.matmul(
                                    sp[:, ii, :],
                                    lhsT=khT[0:DH, h, i * 128 : (i + 1) * 128],
                                    rhs=qT[0:DH, h, :],
                                    start=True, stop=True,
                                )
                            nc.scalar.activation(
                                pbuf[:].rearrange("p a b -> p (a b)"),
                                sp[:].rearrange("p a b -> p (a b)"),
                                AF.Exp,
                            )
                            for ii in range(2):
                                i = gg * 2 + ii
                                nc.tensor.matmul(
                                    pv[:],
                                    lhsT=vaug[:, i, h, :],
                                    rhs=pbuf[:, ii, :],
                                    start=(i == 0), stop=(i == 15),
                                    skip_group_check=True,
                                )
                        # normalize: row DH of pv is the softmax denominator
                        nrm = p2o.tile([97, TQ], f32, tag="nrm")
                        nc.vector.reciprocal(nrm[96:97, :], pv[96:97, :])
                        nc.gpsimd.dma_start(rec_dram[j, h, :], nrm[96:97, :])
                        bc = nrm[0:DH, :]
                        nc.gpsimd.dma_start(
                            bc,
                            APX(rec_dram[j, h, :], [[0, DH], [1, TQ]]),
                        )
                        outT = p2o.tile([DH, TQ], bf16, tag="outT")
                        nc.vector.tensor_tensor(outT[:], pv[0:DH, :], bc, op=ALU.mult)
                        # repack head rows into 128-row proj blocks (SBUF->SBUF DMA)
                        r0 = h * DH
                        cb0, off0 = divmod(r0, 128)
                        n0 = min(DH, 128 - off0)
                        nc.gpsimd.dma_start(
                            proj_in[off0 : off0 + n0, cb0, :], outT[0:n0, :]
                        )
                        if n0 < DH:
                            nc.gpsimd.dma_start(
                                proj_in[0 : DH - n0, cb0 + 1, :], outT[n0:DH, :]
                            )

                    # output projection for this q-chunk
                    for ts in range(4):
                        for e in range(C // ECH):
                            yp = yps.tile([128, ECH], f32, tag="yp")
                            for cb in range(PCB):
                                rows = 128 if cb < PCB - 1 else CPC - 128 * (PCB - 1)
                                nc.tensor.matmul(
                                    yp[:],
                                    lhsT=proj_in[0:rows, cb,
                                                 ts * 128 : (ts + 1) * 128],
                                    rhs=wproj[0:rows, cb,
                                              e * ECH : (e + 1) * ECH],
                                    start=(cb == 0), stop=(cb == PCB - 1),
                                )
                            ysb = p2o.tile([128, ECH], f32, tag="ysb")
                            nc.scalar.copy(ysb[:], yp[:])
                            nc.sync.dma_start(
                                y_d[j * TQ + ts * 128 : j * TQ + (ts + 1) * 128,
                                    e * ECH : (e + 1) * ECH],
                                ysb[:],
                            )

    nc.compile()
    _NC = nc
    return nc


# -------------------------------------------------------------- host prep
def _prep_shards(x, w_qkv, w_proj, q_norm_w, k_norm_w):
    inv_freq = 1.0 / (THETA ** (np.arange(HALF, dtype=np.float32) / HALF))
    ang = np.arange(N, dtype=np.float32)[:, None] * inv_freq[None, :]
    cos_t, sin_t = np.cos(ang), np.sin(ang)  # [N, 36]

    def rope_tabs(w):
        # cos2[t, j] = cos(ang) * w[j] (both halves); sin2s = [-sin, +sin] * w
        c2 = np.concatenate([cos_t * w[:HALF], cos_t * w[HALF:]], axis=1)
        s2 = np.concatenate([-sin_t * w[:HALF], sin_t * w[HALF:]], axis=1)
        tile_form = lambda a: np.ascontiguousarray(
            a.reshape(NT, 128, DH).transpose(1, 0, 2)
        ).astype(np.float32)
        return tile_form(c2), tile_form(s2)

    cq, sq_ = rope_tabs(np.asarray(q_norm_w, np.float32))
    ck, sk = rope_tabs(np.asarray(k_norm_w, np.float32))

    xTs = []
    for b in range(B):
        xt = np.ascontiguousarray(x[b].T)  # [1152, 2048]
        xt = xt.reshape(NCCH, 128, NT, 128).transpose(1, 2, 0, 3)
        xTs.append(np.ascontiguousarray(xt).astype(_BF16))

    in_maps = []
    for core in range(8):
        b, g = divmod(core, 2)
        h0 = g * HPC
        rq = w_qkv[h0 * DH : h0 * DH + CPC]                     # [576, 1152]
        rk = w_qkv[C + h0 * DH : C + h0 * DH + CPC]
        rv = w_qkv[2 * C + h0 * DH : 2 * C + h0 * DH + CPC]
        wk = np.concatenate([rq, rk, rv], axis=0).T             # [1152, 1728]
        wk = wk.reshape(NCCH, 128, 3 * CPC).transpose(1, 0, 2)
        wk = np.ascontiguousarray(wk).astype(_BF16)

        wp = w_proj[:, g * CPC : (g + 1) * CPC].T               # [576, 1152]
        wp = np.concatenate(
            [wp, np.zeros((PCB * 128 - CPC, C), np.float32)], axis=0
        )
        wp = wp.reshape(PCB, 128, C).transpose(1, 0, 2)
        wp = np.ascontiguousarray(wp).astype(_BF16)

        in_maps.append({
            "xT": xTs[b], "wqkv": wk, "wproj": wp,
            "cosq": cq, "sinq": sq_, "cosk": ck, "sink": sk,
        })
    return in_maps


def kernel(x, w_qkv, w_proj, b_proj, q_norm_w, k_norm_w):
    x = np.asarray(x, np.float32)
    w_qkv = np.asarray(w_qkv, np.float32)
    w_proj = np.asarray(w_proj, np.float32)
    b_proj = np.asarray(b_proj, np.float32)

    nc = _build()
    from concourse.bass_utils import run_bass_kernel_spmd

    in_maps = _prep_shards(x, w_qkv, w_proj, q_norm_w, k_norm_w)
    res = run_bass_kernel_spmd(nc, in_maps, core_ids=list(range(8)))
    y = np.empty((B, N, C), np.float32)
    for b in range(B):
        y[b] = res.results[2 * b]["y"] + res.results[2 * b + 1]["y"] + b_proj
    return y
